# revision 11
# baseline (speedup 1.0000x reference)
"""TRN2 Bass kernel for nn_CIN (2-layer Compressed Interaction Network).

Reference computation (per sample b):
  inter0[(p,q),d] = xe[b,p,d] * xe[b,q,d]          (F=39 fields, D=16)
  x1[h,d]  = sum_{p,q} W0[h, p*39+q] * inter0[(p,q),d]   (h=128)
  out0[h]  = sum_d x1[h,d]
  out1[h]  = sum_{i,j} W1[h,i,j] * G[i,j],  G[i,j] = sum_d x1[i,d]*xe[b,j,d]
  out = concat(out0, out1)    -> [B, 256]

v2 strategy (8-core data parallel, 256 samples/core):
  * Symmetrized layer 0 (780 unique pairs as 7 chunks of 117 = 3x39 wrapped
    bands); host ships the row-gathered B operand in 8 column slices so the
    DVE multiply starts early.  The elementwise multiply is split DVE
    (chunks 0-5) / Pool (chunk 6).
  * x1 is produced TRANSPOSED on PE: lhsT = 128-column slice of inter,
    rhs = W0 chunk, PSUM-accumulated over the 7 chunks.  The output blocks
    [(8b,16d), h] are exactly the Gram-trick lhsT groups, so no PE
    transposes and just one PSUM->SBUF copy per slice.
  * Input DMAs spread across the 3 DGE queues (SP/ACT/Pool) -- queues do
    not contend in this machine, so input latency is ~1/3 serial time.
  * Layer 1 via the Gram trick with a block-diagonal xe tensor (+ ones
    column producing out0 for free); G PSUM->SBUF copies split ACT/DVE.
  * L1 j-contraction per 64-sample block; output stays [ch, b] on device
    (host does the final transpose); fp16 output, host upcasts.
  * A dummy matmul at t~0.3us pins pe_busy_start early so all real PE work
    runs at the full 2.4 GHz p-state.
"""

import sys

sys.path.insert(0, "/opt/trn_rl_repo")

import numpy as np
import ml_dtypes

F16 = ml_dtypes.float16 if hasattr(ml_dtypes, "float16") else np.float16

NUM_FIELD = 39
H = 128            # CIN layer width (both layers)
D = 16             # embed dim
BATCH = 2048
NCORES = 8
B_LOC = BATCH // NCORES          # 256
BD = B_LOC * D                   # 4096 columns, b-major / d-minor
NE = 8                           # bd column slices
ECOLS = BD // NE                 # 512 cols = 32 samples
NPAIR = 780                      # unique (p<=q) pairs
CS = 117                         # chunk rows (3*39: A-operand periodic)
NCH = 7                          # ceil(780/117); last chunk zero-padded
NPAD = CS * NCH                  # 819
NGRP = B_LOC // 8                # 32 groups of 8 samples
GW = 40                          # 39 fields + ones column
GQ = 8 * GW * 8                  # 2560 cols of G per quarter (8 groups)
GCOLS = NGRP * 8 * GW            # 10240


def _pairs():
    """Wrapped-band enumeration of the 780 unique pairs."""
    ps, qs = [], []
    for p in range(NUM_FIELD):           # band 0: diagonal
        ps.append(p); qs.append(p)
    for k in range(1, 20):               # bands 1..19
        for p in range(NUM_FIELD):
            ps.append(p); qs.append((p + k) % NUM_FIELD)
    return np.array(ps), np.array(qs)


_P_IDX, _Q_IDX = _pairs()

_COMPILED = None

# L1 sample blocks: (b_offset, n_samples, quarter) -- each block's G columns
# live inside one per-quarter gsb tile.
L1_BLOCKS = [(0, 64, 0), (64, 64, 1), (128, 64, 2), (192, 32, 3), (224, 32, 3)]


def _build_module(debug_taps=False):
    import concourse.bass as bass
    import concourse.bacc as bacc
    import concourse.mybir as mybir
    from concourse import tile

    f32 = mybir.dt.float32
    f16 = mybir.dt.float16

    nc = bacc.Bacc("TRN2", target_bir_lowering=False, debug=False)

    taps = {}
    if debug_taps:
        taps["x1tall"] = nc.dram_tensor("dbg_x1tall", [128, NGRP * 128], f16,
                                        kind="ExternalOutput")
        taps["gsb"] = nc.dram_tensor("dbg_gsb", [128, GCOLS], f16,
                                     kind="ExternalOutput")

    # ---- DRAM parameters (per-core shards prepared host-side) ----
    B8 = nc.dram_tensor("B8", [NE, CS, NCH, ECOLS], f16, kind="ExternalInput")
    XT3 = nc.dram_tensor("XT3", [CS, BD], f16, kind="ExternalInput")
    CONSTA = nc.dram_tensor("CONSTA", [128, NCH * H], f16, kind="ExternalInput")
    CONSTB = nc.dram_tensor("CONSTB", [128, NUM_FIELD * H], f16, kind="ExternalInput")
    BDX = nc.dram_tensor("BDX", [128, GCOLS], f16, kind="ExternalInput")
    # [ch-part, 0|1, b] : row-major per partition; host transposes to [b, ch]
    out = nc.dram_tensor("out", [128, 2, B_LOC], f16, kind="ExternalOutput")

    with tile.TileContext(nc) as tc:
        with tc.tile_pool(name="const", bufs=1) as cpool, \
             tc.tile_pool(name="b", bufs=8) as bpool, \
             tc.tile_pool(name="inter", bufs=2) as ipool, \
             tc.tile_pool(name="psA", bufs=1, space="PSUM") as psA, \
             tc.tile_pool(name="psB", bufs=1, space="PSUM") as psB:

            # ---- SBUF tiles ----
            warm = cpool.tile([128, 16], f16, tag="warm")
            xtx3 = cpool.tile([CS, BD], f16, tag="xtx3")
            ca = cpool.tile([128, NCH * H], f16, tag="ca")
            w0t = ca[:].rearrange("p (c h) -> p c h", c=NCH)
            bdx = cpool.tile([128, GCOLS], f16, tag="bdx")
            w1tt = cpool.tile([128, NUM_FIELD * H], f16, tag="w1tt")
            w1t = w1tt[:].rearrange("p (j h) -> p j h", j=NUM_FIELD)
            x1tall = cpool.tile([128, NGRP, 128], f16, tag="x1tall")
            gsb_qs = [cpool.tile([128, GQ], f16, tag=f"gsb{q}", name=f"gsb{q}")
                      for q in range(4)]
            g_rs = [g[:].rearrange("p (b j) -> p b j", j=GW) for g in gsb_qs]
            outsb = cpool.tile([128, 2, B_LOC], f16, tag="outsb")

            b_ts = [bpool.tile([CS, NCH, ECOLS], f16, tag="b_t", name=f"b{e}")
                    for e in range(NE)]
            inter_ts = [ipool.tile([CS, NCH, ECOLS], f16, tag="inter", name=f"i{e}")
                        for e in range(NE)]

            # ---- PE warmup: pin pe_busy_start near t=0 ----
            nc.vector.memset(warm[:], 0.0)
            warmps = psA.tile([128, 4, 128], f32, tag="x1tps1", name="warmps")
            nc.tensor.matmul(warmps[0:16, 0, 0:16], warm[:], warm[:],
                             start=True, stop=True)

            # ---- DMA dispatch (3 parallel queues: SP / ACT / Pool) ----
            # SP: early-critical stream.  ACT: short list, frees early for
            # copies.  Pool (SWDGE, ~2.7us/DMA overhead): non-urgent bulk.
            nc.sync.dma_start(xtx3[:, 0:2 * ECOLS], XT3[:, 0:2 * ECOLS])
            nc.sync.dma_start(b_ts[0][:, :, 0:256], B8[0][:, :, 0:256])
            nc.sync.dma_start(b_ts[0][:, :, 256:512], B8[0][:, :, 256:512])
            nc.sync.dma_start(xtx3[:, 2 * ECOLS:BD], XT3[:, 2 * ECOLS:BD])
            nc.sync.dma_start(b_ts[2][:], B8[2])
            nc.sync.dma_start(b_ts[4][:], B8[4])
            nc.sync.dma_start(b_ts[5][:], B8[5])
            nc.sync.dma_start(b_ts[7][:], B8[7])
            nc.scalar.dma_start(ca[:], CONSTA[:])
            nc.scalar.dma_start(b_ts[1][:], B8[1])
            nc.scalar.dma_start(w1tt[:], CONSTB[:])
            nc.scalar.dma_start(b_ts[6][:], B8[6])
            nc.gpsimd.dma_start(b_ts[3][:], B8[3])
            nc.gpsimd.dma_start(bdx[:, 0:2 * GQ], BDX[:, 0:2 * GQ])
            nc.gpsimd.dma_start(bdx[:, 2 * GQ:GCOLS], BDX[:, 2 * GQ:GCOLS])

            out1ps = psB.tile([128, B_LOC], f32, tag="out1ps")

            def emit_g_round(q, r):
                gps = psB.tile([128, 2, 512], f32, tag=f"gps{r % 2}",
                               name=f"gps{q}_{r}")
                for gl in range(2):
                    gi = q * 8 + r * 2 + gl
                    nc.tensor.matmul(
                        gps[:, gl, 0:GW * 8],
                        x1tall[:, gi, :],
                        bdx[:, gi * GW * 8:(gi + 1) * GW * 8],
                        start=True, stop=True,
                    )
                li = r * 2 * GW * 8
                dst = gsb_qs[q][:, li:li + 2 * GW * 8] \
                    .rearrange("p (g n) -> p g n", g=2)
                # PSUM->SBUF copy engine: DVE only for q3 rounds 2/3
                if q == 3 and r >= 2:
                    nc.vector.tensor_copy(dst, gps[:, :, 0:GW * 8])
                else:
                    nc.scalar.copy(dst, gps[:, :, 0:GW * 8])

            def emit_out0(q):
                # out0 rows for this quarter's 64 samples from the ones column
                nc.vector.tensor_copy(outsb[:, 0, q * 64:(q + 1) * 64],
                                      g_rs[q][:, :, GW - 1])

            def emit_l1_block(bi):
                boff, bn, q = L1_BLOCKS[bi]
                lo = boff - 64 * q           # sample offset inside the quarter
                for j in range(NUM_FIELD):
                    nc.tensor.matmul(
                        out1ps[:, boff:boff + bn],
                        w1t[:, j, :],
                        g_rs[q][:, lo:lo + bn, j],
                        start=(j == 0), stop=(j == NUM_FIELD - 1),
                    )

            # ---- main pipeline ----
            for e in range(NE):
                # elementwise products: DVE chunks 0-5, Pool chunk 6
                lo = e * ECOLS
                a6 = xtx3[:, lo:lo + ECOLS].unsqueeze(1) \
                    .broadcast_to([CS, NCH - 1, ECOLS])
                if e == 0:
                    for hf in range(2):
                        s = slice(hf * 256, (hf + 1) * 256)
                        a6h = xtx3[:, lo + hf * 256:lo + (hf + 1) * 256] \
                            .unsqueeze(1).broadcast_to([CS, NCH - 1, 256])
                        nc.vector.tensor_mul(inter_ts[0][:, 0:NCH - 1, s],
                                             a6h, b_ts[0][:, 0:NCH - 1, s])
                        nc.gpsimd.tensor_mul(inter_ts[0][:, NCH - 1, s],
                                             xtx3[:, lo + hf * 256:lo + (hf + 1) * 256],
                                             b_ts[0][:, NCH - 1, s])
                else:
                    nc.vector.tensor_mul(inter_ts[e][:, 0:NCH - 1, :],
                                         a6, b_ts[e][:, 0:NCH - 1, :])
                    nc.gpsimd.tensor_mul(inter_ts[e][:, NCH - 1, :],
                                         xtx3[:, lo:lo + ECOLS],
                                         b_ts[e][:, NCH - 1, :])

                # x1 transposed: out [(8b,16d)-block, h], accumulated over chunks
                x1tps = psA.tile([128, 4, 128], f32, tag=f"x1tps{e % 2}",
                                 name=f"x1tps{e}")
                for blk in range(4):
                    for c in range(NCH):
                        nc.tensor.matmul(
                            x1tps[:, blk, :],
                            inter_ts[e][:, c, blk * 128:(blk + 1) * 128],
                            w0t[0:CS, c, :],
                            start=(c == 0), stop=(c == NCH - 1),
                        )
                if e in (2, 4, 6):
                    emit_g_round(e // 2 - 1, 2)
                    emit_g_round(e // 2 - 1, 3)
                if e == 4:
                    emit_l1_block(0)
                if e == 5:
                    emit_l1_block(1)

                if e < 2:
                    nc.vector.tensor_copy(x1tall[:, e * 4:(e + 1) * 4, :],
                                          x1tps[:])
                else:
                    nc.scalar.copy(x1tall[:, e * 4:(e + 1) * 4, :], x1tps[:])

                if e % 2 == 1:
                    q = e // 2
                    emit_g_round(q, 0)
                    emit_g_round(q, 1)

            emit_g_round(3, 2)
            emit_g_round(3, 3)
            emit_out0(0)
            emit_out0(1)
            emit_out0(2)
            emit_out0(3)
            emit_l1_block(2)
            emit_l1_block(3)
            emit_l1_block(4)
            nc.vector.tensor_copy(outsb[:, 1, :], out1ps[:])
            nc.sync.dma_start(out[:], outsb[:])

            if debug_taps:
                nc.sync.dma_start(
                    taps["x1tall"][:],
                    x1tall[:].rearrange("p g n -> p (g n)"))
                for q in range(4):
                    nc.sync.dma_start(taps["gsb"][:, q * GQ:(q + 1) * GQ],
                                      gsb_qs[q][:])

    nc.compile()
    return nc


def _host_prep(x_emb, W0, W1):
    """Build per-core input maps (layout/dtype repacking only)."""
    maps = []
    # weights: symmetrized / repacked, shared by all cores
    W0m = W0.reshape(H, NUM_FIELD, NUM_FIELD)
    W0sym = W0m[:, _P_IDX, _Q_IDX] + np.where(
        (_P_IDX != _Q_IDX)[None, :], W0m[:, _Q_IDX, _P_IDX], 0.0
    )                                            # [H, 780]
    W0p = np.zeros((H, NPAD), np.float32)
    W0p[:, :NPAIR] = W0sym
    w0t = np.zeros((128, NCH, H), np.float32)
    w0t[0:CS] = W0p.T.reshape(NCH, CS, H).transpose(1, 0, 2)
    w0t = w0t.astype(F16)

    w1t = np.ascontiguousarray(
        W1.reshape(H, H, NUM_FIELD).transpose(1, 2, 0)
    ).astype(F16)                                # [i, j, h]

    consta = np.ascontiguousarray(w0t.reshape(128, -1)).astype(F16)
    constb = np.ascontiguousarray(w1t.reshape(128, -1)).astype(F16)

    for core in range(NCORES):
        xe = x_emb[core * B_LOC:(core + 1) * B_LOC]          # [256, 39, 16]
        xT = np.ascontiguousarray(xe.transpose(1, 0, 2)).reshape(NUM_FIELD, BD)
        xT16 = xT.astype(F16)

        Bm = xT16[_Q_IDX]                                    # [780, 4096]
        Bp = np.zeros((NPAD, BD), F16)
        Bp[:NPAIR] = Bm
        B8 = np.ascontiguousarray(
            Bp.reshape(NCH, CS, NE, ECOLS).transpose(2, 1, 0, 3))
        xt3 = np.tile(xT16, (3, 1))                          # [117, 4096]

        # block-diagonal xe (+ ones column), [128=(b8,d), 32grp*8b*40]
        bdx = np.zeros((128, NGRP, 8, GW), np.float32)
        xe_t = xe.transpose(0, 2, 1)                         # [b, d, j]
        for bb in range(8):
            rows = slice(bb * D, (bb + 1) * D)
            bdx[rows, :, bb, 0:NUM_FIELD] = (
                xe_t[bb::8].transpose(1, 0, 2))              # [d, g, j]
            bdx[rows, :, bb, GW - 1] = 1.0
        bdx = bdx.reshape(128, GCOLS).astype(F16)

        maps.append({
            "B8": B8, "XT3": xt3, "BDX": bdx,
            "CONSTA": consta, "CONSTB": constb,
        })
    return maps


def kernel(x_emb, W0, W1, _trace=False, _trace_kwargs=None):
    global _COMPILED
    if _COMPILED is None:
        _COMPILED = _build_module()
    nc = _COMPILED

    from concourse.bass_utils import run_bass_kernel_spmd

    in_maps = _host_prep(np.asarray(x_emb, np.float32),
                         np.asarray(W0, np.float32),
                         np.asarray(W1, np.float32))
    kw = {}
    if _trace:
        kw["trace"] = True
        kw.update(_trace_kwargs or {})
    res = run_bass_kernel_spmd(nc, in_maps, list(range(NCORES)), **kw)
    parts = []
    for i in range(NCORES):
        o = res.results[i]["out"].astype(np.float32)         # [128, 2, 256]
        parts.append(np.concatenate([o[:, 0, :].T, o[:, 1, :].T], axis=1))
    outp = np.concatenate(parts, axis=0)
    if _trace:
        return outp, res
    return outp


# revision 13
# speedup vs baseline: 1.2420x; 1.2420x over previous
"""TRN2 Bass kernel for nn_CIN (2-layer Compressed Interaction Network).

Reference computation (per sample b):
  inter0[(p,q),d] = xe[b,p,d] * xe[b,q,d]          (F=39 fields, D=16)
  x1[h,d]  = sum_{p,q} W0[h, p*39+q] * inter0[(p,q),d]   (h=128)
  out0[h]  = sum_d x1[h,d]
  out1[h]  = sum_{i,j} W1[h,i,j] * G[i,j],  G[i,j] = sum_d x1[i,d]*xe[b,j,d]
  out = concat(out0, out1)    -> [B, 256]

v2 strategy (8-core data parallel, 256 samples/core):
  * Symmetrized layer 0 (780 unique pairs as 7 chunks of 117 = 3x39 wrapped
    bands); host ships the row-gathered B operand in 8 column slices so the
    DVE multiply starts early.  The elementwise multiply is split DVE
    (chunks 0-5) / Pool (chunk 6).
  * x1 is produced TRANSPOSED on PE: lhsT = 128-column slice of inter,
    rhs = W0 chunk, PSUM-accumulated over the 7 chunks.  The output blocks
    [(8b,16d), h] are exactly the Gram-trick lhsT groups, so no PE
    transposes and just one PSUM->SBUF copy per slice.
  * Input DMAs spread across the 3 DGE queues (SP/ACT/Pool) -- queues do
    not contend in this machine, so input latency is ~1/3 serial time.
  * Layer 1 via the Gram trick with a block-diagonal xe tensor (+ ones
    column producing out0 for free); G PSUM->SBUF copies split ACT/DVE.
  * L1 j-contraction per 64-sample block; output stays [ch, b] on device
    (host does the final transpose); fp16 output, host upcasts.
  * A dummy matmul at t~0.3us pins pe_busy_start early so all real PE work
    runs at the full 2.4 GHz p-state.
"""

import sys

sys.path.insert(0, "/opt/trn_rl_repo")

import numpy as np
import ml_dtypes

F16 = ml_dtypes.float16 if hasattr(ml_dtypes, "float16") else np.float16

NUM_FIELD = 39
H = 128            # CIN layer width (both layers)
D = 16             # embed dim
BATCH = 2048
NCORES = 8
B_LOC = BATCH // NCORES          # 256
BD = B_LOC * D                   # 4096 columns, b-major / d-minor
NE = 8                           # bd column slices
ECOLS = BD // NE                 # 512 cols = 32 samples
NPAIR = 780                      # unique (p<=q) pairs
CS = 117                         # chunk rows (3*39: A-operand periodic)
NCH = 7                          # ceil(780/117); last chunk zero-padded
NPAD = CS * NCH                  # 819
NGRP = B_LOC // 8                # 32 groups of 8 samples
GW = 40                          # 39 fields + ones column
GQ = 8 * GW * 8                  # 2560 cols of G per quarter (8 groups)
GCOLS = NGRP * 8 * GW            # 10240


def _pairs():
    """Wrapped-band enumeration of the 780 unique pairs."""
    ps, qs = [], []
    for p in range(NUM_FIELD):           # band 0: diagonal
        ps.append(p); qs.append(p)
    for k in range(1, 20):               # bands 1..19
        for p in range(NUM_FIELD):
            ps.append(p); qs.append((p + k) % NUM_FIELD)
    return np.array(ps), np.array(qs)


_P_IDX, _Q_IDX = _pairs()

_COMPILED = None

# L1 sample blocks: (b_offset, n_samples, quarter) -- each block's G columns
# live inside one per-quarter gsb tile.
L1_BLOCKS = [(0, 64, 0), (64, 64, 1), (128, 64, 2), (192, 32, 3), (224, 32, 3)]


def _build_module(debug_taps=False):
    import concourse.bass as bass
    import concourse.bacc as bacc
    import concourse.mybir as mybir
    from concourse import tile

    f32 = mybir.dt.float32
    f16 = mybir.dt.float16

    nc = bacc.Bacc("TRN2", target_bir_lowering=False, debug=False)

    taps = {}
    if debug_taps:
        taps["x1tall"] = nc.dram_tensor("dbg_x1tall", [128, NGRP * 128], f16,
                                        kind="ExternalOutput")
        taps["gsb"] = nc.dram_tensor("dbg_gsb", [128, GCOLS], f16,
                                     kind="ExternalOutput")

    # ---- DRAM parameters (per-core shards prepared host-side) ----
    B8 = nc.dram_tensor("B8", [NE, CS, NCH, ECOLS], f16, kind="ExternalInput")
    XT3 = nc.dram_tensor("XT3", [CS, BD], f16, kind="ExternalInput")
    CONSTA = nc.dram_tensor("CONSTA", [128, NCH * H], f16, kind="ExternalInput")
    CONSTB = nc.dram_tensor("CONSTB", [128, NUM_FIELD * H], f16, kind="ExternalInput")
    BDX = nc.dram_tensor("BDX", [128, GCOLS], f16, kind="ExternalInput")
    # [ch-part, 0|1, b] : row-major per partition; host transposes to [b, ch]
    out = nc.dram_tensor("out", [128, 2, B_LOC], f16, kind="ExternalOutput")

    with tile.TileContext(nc) as tc:
        with tc.tile_pool(name="const", bufs=1) as cpool, \
             tc.tile_pool(name="b", bufs=1) as bpool, \
             tc.tile_pool(name="inter", bufs=2) as ipool, \
             tc.tile_pool(name="psA", bufs=1, space="PSUM") as psA, \
             tc.tile_pool(name="psB", bufs=1, space="PSUM") as psB:

            # ---- SBUF tiles ----
            warm = cpool.tile([128, 16], f16, tag="warm")
            xtx3 = cpool.tile([CS, BD], f16, tag="xtx3")
            ca = cpool.tile([128, NCH * H], f16, tag="ca")
            w0t = ca[:].rearrange("p (c h) -> p c h", c=NCH)
            bdx = cpool.tile([128, GCOLS], f16, tag="bdx")
            w1tt = cpool.tile([128, NUM_FIELD * H], f16, tag="w1tt")
            w1t = w1tt[:].rearrange("p (j h) -> p j h", j=NUM_FIELD)
            x1tall = cpool.tile([128, NGRP, 128], f16, tag="x1tall")
            gsb_qs = [cpool.tile([128, GQ], f16, tag=f"gsb{q}", name=f"gsb{q}")
                      for q in range(4)]
            g_rs = [g[:].rearrange("p (b j) -> p b j", j=GW) for g in gsb_qs]
            outsb = cpool.tile([128, 2, B_LOC], f16, tag="outsb")

            bigB = bpool.tile([CS, NE, NCH, ECOLS], f16, tag="bigB")
            inter_ts = [ipool.tile([CS, NCH, ECOLS], f16, tag="inter", name=f"i{e}")
                        for e in range(NE)]

            # ---- PE warmup: pin pe_busy_start near t=0 ----
            nc.vector.memset(warm[:], 0.0)
            warmps = psA.tile([128, 4, 128], f32, tag="x1tps1", name="warmps")
            nc.tensor.matmul(warmps[0:16, 0, 0:16], warm[:], warm[:],
                             start=True, stop=True)

            # ---- DMA dispatch (3 parallel queues: SP / ACT / Pool) ----
            # SP: early-critical stream.  ACT: frees by ~11us for copies.
            # Pool (SWDGE, ~2.7us/DMA gap): bdx halves + one B slice.
            nc.sync.dma_start(xtx3[:, 0:2 * ECOLS], XT3[:, 0:2 * ECOLS])
            nc.sync.dma_start(bigB[:, 0, :, 0:256], B8[0][:, :, 0:256])
            nc.sync.dma_start(bigB[:, 0, :, 256:512], B8[0][:, :, 256:512])
            nc.sync.dma_start(xtx3[:, 2 * ECOLS:BD], XT3[:, 2 * ECOLS:BD])
            nc.sync.dma_start(bigB[:, 2], B8[2])
            nc.sync.dma_start(bigB[:, 4], B8[4])
            nc.sync.dma_start(bigB[:, 5], B8[5])
            nc.scalar.dma_start(ca[:], CONSTA[:])
            nc.scalar.dma_start(bigB[:, 1], B8[1])
            nc.scalar.dma_start(w1tt[:], CONSTB[:])
            nc.scalar.dma_start(bigB[:, 6:8], B8[6:8].rearrange("e c x y -> c e x y"))
            nc.gpsimd.dma_start(bdx[:, 0:2 * GQ], BDX[:, 0:2 * GQ])
            nc.gpsimd.dma_start(bigB[:, 3], B8[3])
            nc.gpsimd.dma_start(bdx[:, 2 * GQ:GCOLS], BDX[:, 2 * GQ:GCOLS])

            out1ps = psB.tile([128, B_LOC], f32, tag="out1ps")

            def emit_g_round(q, r):
                gps = psB.tile([128, 2, 512], f32, tag=f"gps{r % 2}",
                               name=f"gps{q}_{r}")
                for gl in range(2):
                    gi = q * 8 + r * 2 + gl
                    nc.tensor.matmul(
                        gps[:, gl, 0:GW * 8],
                        x1tall[:, gi, :],
                        bdx[:, gi * GW * 8:(gi + 1) * GW * 8],
                        start=True, stop=True,
                    )
                li = r * 2 * GW * 8
                dst = gsb_qs[q][:, li:li + 2 * GW * 8] \
                    .rearrange("p (g n) -> p g n", g=2)
                # PSUM->SBUF copy engine: DVE for late-quarter rounds 2/3
                if q >= 2 and r >= 2:
                    nc.vector.tensor_copy(dst, gps[:, :, 0:GW * 8])
                else:
                    nc.scalar.copy(dst, gps[:, :, 0:GW * 8])

            def emit_out0(q):
                # out0 rows for this quarter's 64 samples from the ones column
                nc.scalar.copy(outsb[:, 0, q * 64:(q + 1) * 64],
                               g_rs[q][:, :, GW - 1])

            def emit_l1_block(bi):
                boff, bn, q = L1_BLOCKS[bi]
                lo = boff - 64 * q           # sample offset inside the quarter
                for j in range(NUM_FIELD):
                    nc.tensor.matmul(
                        out1ps[:, boff:boff + bn],
                        w1t[:, j, :],
                        g_rs[q][:, lo:lo + bn, j],
                        start=(j == 0), stop=(j == NUM_FIELD - 1),
                    )

            # ---- main pipeline ----
            for e in range(NE):
                # elementwise products: DVE chunks 0-5, Pool chunk 6
                lo = e * ECOLS
                a7 = xtx3[:, lo:lo + ECOLS].unsqueeze(1) \
                    .broadcast_to([CS, NCH, ECOLS])
                if e == 0:
                    for hf in range(2):
                        s = slice(hf * 256, (hf + 1) * 256)
                        a7h = xtx3[:, lo + hf * 256:lo + (hf + 1) * 256] \
                            .unsqueeze(1).broadcast_to([CS, NCH, 256])
                        nc.vector.tensor_mul(inter_ts[0][:, :, s],
                                             a7h, bigB[:, 0, :, s])
                else:
                    nc.vector.tensor_mul(inter_ts[e][:], a7, bigB[:, e])

                # x1 transposed: out [(8b,16d)-block, h], accumulated over chunks
                x1tps = psA.tile([128, 4, 128], f32, tag=f"x1tps{e % 2}",
                                 name=f"x1tps{e}")
                for blk in range(4):
                    for c in range(NCH):
                        nc.tensor.matmul(
                            x1tps[:, blk, :],
                            inter_ts[e][:, c, blk * 128:(blk + 1) * 128],
                            w0t[0:CS, c, :],
                            start=(c == 0), stop=(c == NCH - 1),
                        )
                if e in (2, 4, 6):
                    emit_g_round(e // 2 - 1, 2)
                    emit_g_round(e // 2 - 1, 3)
                if e == 4:
                    emit_l1_block(0)
                if e == 5:
                    emit_l1_block(1)

                if e <= 4:
                    nc.vector.tensor_copy(x1tall[:, e * 4:(e + 1) * 4, :],
                                          x1tps[:])
                else:
                    nc.scalar.copy(x1tall[:, e * 4:(e + 1) * 4, :], x1tps[:])

                if e % 2 == 1:
                    q = e // 2
                    emit_g_round(q, 0)
                    emit_g_round(q, 1)

            emit_g_round(3, 2)
            emit_g_round(3, 3)
            emit_out0(0)
            emit_out0(1)
            emit_out0(2)
            emit_out0(3)
            emit_l1_block(2)
            emit_l1_block(3)
            emit_l1_block(4)
            nc.vector.tensor_copy(outsb[:, 1, :], out1ps[:])
            nc.sync.dma_start(out[:], outsb[:])

            if debug_taps:
                nc.sync.dma_start(
                    taps["x1tall"][:],
                    x1tall[:].rearrange("p g n -> p (g n)"))
                for q in range(4):
                    nc.sync.dma_start(taps["gsb"][:, q * GQ:(q + 1) * GQ],
                                      gsb_qs[q][:])

    nc.compile()
    return nc


def _host_prep(x_emb, W0, W1):
    """Build per-core input maps (layout/dtype repacking only)."""
    maps = []
    # weights: symmetrized / repacked, shared by all cores
    W0m = W0.reshape(H, NUM_FIELD, NUM_FIELD)
    W0sym = W0m[:, _P_IDX, _Q_IDX] + np.where(
        (_P_IDX != _Q_IDX)[None, :], W0m[:, _Q_IDX, _P_IDX], 0.0
    )                                            # [H, 780]
    W0p = np.zeros((H, NPAD), np.float32)
    W0p[:, :NPAIR] = W0sym
    w0t = np.zeros((128, NCH, H), np.float32)
    w0t[0:CS] = W0p.T.reshape(NCH, CS, H).transpose(1, 0, 2)
    w0t = w0t.astype(F16)

    w1t = np.ascontiguousarray(
        W1.reshape(H, H, NUM_FIELD).transpose(1, 2, 0)
    ).astype(F16)                                # [i, j, h]

    consta = np.ascontiguousarray(w0t.reshape(128, -1)).astype(F16)
    constb = np.ascontiguousarray(w1t.reshape(128, -1)).astype(F16)

    for core in range(NCORES):
        xe = x_emb[core * B_LOC:(core + 1) * B_LOC]          # [256, 39, 16]
        xT = np.ascontiguousarray(xe.transpose(1, 0, 2)).reshape(NUM_FIELD, BD)
        xT16 = xT.astype(F16)

        Bm = xT16[_Q_IDX]                                    # [780, 4096]
        Bp = np.zeros((NPAD, BD), F16)
        Bp[:NPAIR] = Bm
        B8 = np.ascontiguousarray(
            Bp.reshape(NCH, CS, NE, ECOLS).transpose(2, 1, 0, 3))
        xt3 = np.tile(xT16, (3, 1))                          # [117, 4096]

        # block-diagonal xe (+ ones column), [128=(b8,d), 32grp*8b*40]
        bdx = np.zeros((128, NGRP, 8, GW), np.float32)
        xe_t = xe.transpose(0, 2, 1)                         # [b, d, j]
        for bb in range(8):
            rows = slice(bb * D, (bb + 1) * D)
            bdx[rows, :, bb, 0:NUM_FIELD] = (
                xe_t[bb::8].transpose(1, 0, 2))              # [d, g, j]
            bdx[rows, :, bb, GW - 1] = 1.0
        bdx = bdx.reshape(128, GCOLS).astype(F16)

        maps.append({
            "B8": B8, "XT3": xt3, "BDX": bdx,
            "CONSTA": consta, "CONSTB": constb,
        })
    return maps


def kernel(x_emb, W0, W1, _trace=False, _trace_kwargs=None):
    global _COMPILED
    if _COMPILED is None:
        _COMPILED = _build_module()
    nc = _COMPILED

    from concourse.bass_utils import run_bass_kernel_spmd

    in_maps = _host_prep(np.asarray(x_emb, np.float32),
                         np.asarray(W0, np.float32),
                         np.asarray(W1, np.float32))
    kw = {}
    if _trace:
        kw["trace"] = True
        kw.update(_trace_kwargs or {})
    res = run_bass_kernel_spmd(nc, in_maps, list(range(NCORES)), **kw)
    parts = []
    for i in range(NCORES):
        o = res.results[i]["out"].astype(np.float32)         # [128, 2, 256]
        parts.append(np.concatenate([o[:, 0, :].T, o[:, 1, :].T], axis=1))
    outp = np.concatenate(parts, axis=0)
    if _trace:
        return outp, res
    return outp
